# revision 1
# baseline (speedup 1.0000x reference)
"""Trainium2 Bass kernel for nn_IntraCycleMoELayer (MoE routing, 8 cores).

Strategy
--------
The reference computes all E=8 experts densely, but the top-2 gate zeroes all
but 2 experts per batch row.  Real work: for each of B=16 rows, 2 routed
expert MLP blocks + 1 general MLP block = 48 applications of
  LN(gelu_tanh(x @ w1 + b1) @ w2 + b2 + x) * gamma + beta
over [L=512 tokens, D=768] with DFF=3072.

The tiny router is computed on the host (numpy, fp32) when kernel() is called;
the Bass program is built at call time, so the dispatch schedule is baked in
as static data movement.  Each of the 8 cores processes 2 batch rows = 6 jobs
(2 routed + 1 general per row).  The gate coefficient is folded into
gamma/beta host-side (LN output is linear in gamma/beta), so every job is a
plain MLP block and the host only sums per-row outputs at the end.

Per-job device pipeline (all matmul inputs fp16, fp32 PSUM accumulation):
  mm1: h^T[dff,tok] += w1_chunk.T @ x^T      (24x6 matmuls, N=512)
  ACT: h = gelu_tanh(psum + b1) -> SBUF fp16 (per-partition bias)
  mm2: o[tok,d]     += h_chunk.T @ w2        (4x24x2 matmuls, N=512/256)
  DVE: r = o + (x + b2);  LN via bn_stats/bn_aggr; r = (r-mu)*rstd*gamma+beta
  DMA out fp32.

Weight SBUF reuse across jobs with the same expert is baked in when ALL cores
share the dedupe pattern (always true for the "general" pair; true for routed
experts when the routing is uniform across rows, as it is for the graded
inputs where every row routes to the same two experts).
"""
import numpy as np

import concourse.bass as bass
import concourse.mybir as mybir
import concourse.tile as tile
from concourse import bacc
from concourse.bass import ts
from concourse import bass_utils

B, L, D, DFF, DLLM, E, TOPK = 16, 512, 768, 3072, 4096, 8, 2
EPS_GATE = 1e-9
LN_EPS = 1e-5
NCORES = 8
ROWS_PER_CORE = B // NCORES          # 2
JOBS_PER_CORE = ROWS_PER_CORE * (TOPK + 1)  # 6
KC1, MC1 = D // 128, DFF // 128      # 6, 24
KC2, TC = DFF // 128, L // 128       # 24, 4
dt = mybir.dt

_cache = {}  # (n_uniq, tuple(load_uniq)) -> finalized nc


def _router(cycle_numbers, DKP_embeddings, gate_We, gate_Wc, gate_b, gate_Wo,
            gate_bo):
    """Replicate the reference router in fp32 numpy: top-2 indices + gates."""
    h = np.maximum(
        DKP_embeddings @ gate_We + cycle_numbers @ gate_Wc + gate_b, 0.0)
    logits = h @ gate_Wo + gate_bo                       # [B, E]
    idx = np.argsort(-logits, axis=1, kind="stable")[:, :TOPK]
    m = logits.max(axis=1, keepdims=True)
    p = np.exp(logits - m)
    p /= p.sum(axis=1, keepdims=True)
    mask = np.zeros_like(p)
    mask[np.arange(logits.shape[0])[:, None], idx] = 1.0
    gated = p * mask
    gated = gated / (gated.sum(axis=1, keepdims=True) + EPS_GATE)
    return idx, gated


def _build_nc(n_uniq, load_uniq):
    """Build the SPMD per-core program.

    load_uniq[j] is the packed unique-weight-slot index to DMA before job j,
    or None to reuse the previously loaded weights (identical across cores).
    """
    key = (n_uniq, tuple(load_uniq))
    if key in _cache:
        return _cache[key]

    nc = bacc.Bacc("TRN2", target_bir_lowering=False, debug=False)
    w1_d = nc.dram_tensor("w1", [n_uniq, D, DFF], dt.float16, kind="ExternalInput")
    w2_d = nc.dram_tensor("w2", [n_uniq, DFF, D], dt.float16, kind="ExternalInput")
    xT_d = nc.dram_tensor("xT", [ROWS_PER_CORE, D, L], dt.float16, kind="ExternalInput")
    xr_d = nc.dram_tensor("xr", [JOBS_PER_CORE, L, D], dt.float16, kind="ExternalInput")
    b1_d = nc.dram_tensor("b1", [128, JOBS_PER_CORE, MC1], dt.float32, kind="ExternalInput")
    gb_d = nc.dram_tensor("gb", [JOBS_PER_CORE, 2, D], dt.float16, kind="ExternalInput")
    y_d = nc.dram_tensor("y", [JOBS_PER_CORE, L, D], dt.float32, kind="ExternalOutput")

    gelu = mybir.ActivationFunctionType.Gelu_apprx_tanh

    with tile.TileContext(nc) as tc, \
         tc.tile_pool(name="w1p", bufs=2) as w1p, \
         tc.tile_pool(name="w2p", bufs=1) as w2p, \
         tc.tile_pool(name="xtp", bufs=ROWS_PER_CORE) as xtp, \
         tc.tile_pool(name="xrp", bufs=2) as xrp, \
         tc.tile_pool(name="hp", bufs=1) as hp, \
         tc.tile_pool(name="gbp", bufs=2) as gbp, \
         tc.tile_pool(name="rp", bufs=3) as rp, \
         tc.tile_pool(name="sp", bufs=4) as sp, \
         tc.tile_pool(name="cp", bufs=1) as cp, \
         tc.tile_pool(name="php", bufs=4, space="PSUM") as php, \
         tc.tile_pool(name="pop", bufs=2, space="PSUM") as pop:

        from concourse.bass import _add_dep_helper

        eps_t = cp.tile([128, 1], dt.float32)
        nc.vector.memset(eps_t, LN_EPS)

        # all-jobs b1 in one well-shaped DMA (576B/partition lines), early
        b1_all = cp.tile([128, JOBS_PER_CORE, MC1], dt.float32)
        nc.gpsimd.dma_start(b1_all, b1_d[:])

        # PE warmup: ~32 matmuls on zeros so the HAM clock-gate reaches
        # 8/8 while the first weight DMAs are still in flight.
        warm_z = cp.tile([128, 512], dt.float16)
        nc.vector.memset(warm_z, 0.0)
        for _ in range(32):
            wp_t = php.tile([128, L], dt.float32, tag="ph")
            nc.tensor.matmul(wp_t, lhsT=warm_z[:, 0:128], rhs=warm_z,
                             start=True, stop=True)

        # xT row 0 split per k-chunk: first-matmul deps land fast.  Row 1 is
        # loaded later (delayed behind the first matmul, below).
        xT_sb = []
        for r in range(ROWS_PER_CORE):
            t = xtp.tile([128, KC1, L], dt.float16, tag="xT")
            xT_sb.append(t)
        xT_src0 = xT_d[0].rearrange("(ko p) l -> p ko l", p=128)
        for k in range(KC1):
            nc.sync.dma_start(xT_sb[0][:, k, :], xT_src0[:, k, :])

        first_mm = None      # anchor for delaying non-critical head DMAs
        deferred = []        # DMA insts to hook behind first_mm

        w1_sb = w2_sb = None
        for j in range(JOBS_PER_CORE):
            row = j % ROWS_PER_CORE
            if load_uniq[j] is not None:
                u = load_uniq[j]
                # w1 on the critical path: per-(k, half) splits on HWDGE
                w1_sb = w1p.tile([128, KC1, DFF], dt.float16, tag="w1")
                w1_src = w1_d[u].rearrange("(ko p) n -> p ko n", p=128)
                H = DFF // 2
                for k in range(KC1):
                    nc.sync.dma_start(w1_sb[:, k, 0:H], w1_src[:, k, 0:H])
                for k in range(KC1):
                    nc.sync.dma_start(w1_sb[:, k, H:DFF], w1_src[:, k, H:DFF])
                # w2 is needed only after all of mm1: bulk-load via SWDGE
                # (gpsimd) so it does not head-of-line-block w1/xT
                w2_sb = w2p.tile([128, KC2, D], dt.float16, tag="w2")
                w2_src = w2_d[u].rearrange("(ko p) n -> p ko n", p=128)
                for k in range(0, KC2, 6):
                    dma = nc.gpsimd.dma_start(w2_sb[:, k:k + 6, :],
                                              w2_src[:, k:k + 6, :])
                    if j == 0:
                        deferred.append(dma)
            gb_sb = gbp.tile([128, 2, D], dt.float16, tag="gb")
            gb_ap = gb_d[j]
            dma = nc.gpsimd.dma_start(gb_sb, bass.AP(tensor=gb_ap.tensor,
                                                     offset=gb_ap.offset,
                                                     ap=[[0, 128], *gb_ap.ap]))
            if j == 0:
                deferred.append(dma)
            xr_sb = xrp.tile([128, TC, D], dt.float16, tag="xr")
            xr_src = xr_d[j].rearrange("(t p) d -> p t d", p=128)
            for t in range(TC):
                dma = nc.gpsimd.dma_start(xr_sb[:, t, :], xr_src[:, t, :])
                if j == 0:
                    deferred.append(dma)
            if j == 0:
                # remaining xT rows, behind the critical head data
                for r in range(1, ROWS_PER_CORE):
                    src = xT_d[r].rearrange("(ko p) l -> p ko l", p=128)
                    for k in range(KC1):
                        deferred.append(
                            nc.sync.dma_start(xT_sb[r][:, k, :], src[:, k, :]))
            b1_sb = b1_all[:, j, :]

            # mm1 + gelu: h^T [DFF on partitions, tokens free]
            h_sb = hp.tile([128, KC2, L], dt.float16, tag="h")
            for m in range(MC1):
                ph = php.tile([128, L], dt.float32, tag="ph")
                for k in range(KC1):
                    mm = nc.tensor.matmul(ph, lhsT=w1_sb[:, k, ts(m, 128)],
                                          rhs=xT_sb[row][:, k, :],
                                          start=(k == 0), stop=(k == KC1 - 1))
                    if first_mm is None and j == 0 and m == 12 and k == 0:
                        first_mm = mm
                        for dma in deferred:
                            _add_dep_helper(
                                dma.ins, first_mm.ins, sync=True,
                                reason="delay non-critical head DMA")
                nc.scalar.activation(out=h_sb[:, m, :], in_=ph, func=gelu,
                                     bias=b1_sb[:, m:m + 1], scale=1.0)

            # mm2 + residual + LN per 128-token chunk
            for t in range(TC):
                po = pop.tile([128, D], dt.float32, tag="po")
                for k in range(KC2):
                    nc.tensor.matmul(po[:, 0:512], lhsT=h_sb[:, k, ts(t, 128)],
                                     rhs=w2_sb[:, k, 0:512],
                                     start=(k == 0), stop=(k == KC2 - 1))
                    nc.tensor.matmul(po[:, 512:D], lhsT=h_sb[:, k, ts(t, 128)],
                                     rhs=w2_sb[:, k, 512:D],
                                     start=(k == 0), stop=(k == KC2 - 1))
                r_sb = rp.tile([128, D], dt.float32, tag="r")
                nc.vector.tensor_add(r_sb, po, xr_sb[:, t, :])
                stats = sp.tile([128, 3, 6], dt.float32, tag="st")
                for s in range(3):
                    nc.vector.bn_stats(stats[:, s, :], r_sb[:, ts(s, 256)])
                mv = sp.tile([128, 2], dt.float32, tag="mv")
                nc.vector.bn_aggr(mv, stats)
                rstd = sp.tile([128, 1], dt.float32, tag="rstd")
                nc.scalar.activation(out=rstd, in_=mv[:, 1:2],
                                     func=mybir.ActivationFunctionType.Sqrt,
                                     bias=eps_t, scale=1.0)
                nc.vector.reciprocal(rstd, rstd)
                nc.vector.tensor_scalar(out=r_sb, in0=r_sb, scalar1=mv[:, 0:1],
                                        scalar2=rstd,
                                        op0=mybir.AluOpType.subtract,
                                        op1=mybir.AluOpType.mult)
                nc.vector.tensor_mul(r_sb, r_sb, gb_sb[:, 0, :])
                nc.vector.tensor_add(r_sb, r_sb, gb_sb[:, 1, :])
                nc.sync.dma_start(
                    y_d[j].rearrange("(t p) d -> p t d", p=128)[:, t, :], r_sb)

    nc.finalize()
    _cache[key] = nc
    return nc


def kernel(cycle_curve_data, cycle_numbers, DKP_embeddings,
           gate_We, gate_Wc, gate_b, gate_Wo, gate_bo,
           e_w1, e_b1, e_w2, e_b2, e_gamma, e_beta,
           g_w1, g_b1, g_w2, g_b2, g_gamma, g_beta):
    x = np.asarray(cycle_curve_data, dtype=np.float32)
    idx, gated = _router(np.asarray(cycle_numbers, np.float32),
                         np.asarray(DKP_embeddings, np.float32),
                         np.asarray(gate_We, np.float32),
                         np.asarray(gate_Wc, np.float32),
                         np.asarray(gate_b, np.float32),
                         np.asarray(gate_Wo, np.float32),
                         np.asarray(gate_bo, np.float32))

    # Weight sets: 0..E-1 experts, E = general.
    GEN = E
    w1s = {**{e: np.asarray(e_w1[e]) for e in range(E)}, GEN: np.asarray(g_w1)}
    w2s = {**{e: np.asarray(e_w2[e]) for e in range(E)}, GEN: np.asarray(g_w2)}
    b1s = {**{e: np.asarray(e_b1[e]) for e in range(E)}, GEN: np.asarray(g_b1)}
    b2s = {**{e: np.asarray(e_b2[e]) for e in range(E)}, GEN: np.asarray(g_b2)}
    gms = {**{e: np.asarray(e_gamma[e]) for e in range(E)}, GEN: np.asarray(g_gamma)}
    bts = {**{e: np.asarray(e_beta[e]) for e in range(E)}, GEN: np.asarray(g_beta)}

    # Job list per core: rows (2c, 2c+1); order = [(r0,eA),(r1,eA'),(r0,eB),
    # (r1,eB'),(r0,GEN),(r1,GEN)] with each row's routed experts sorted by id
    # to maximize the chance of a core-uniform dedupe pattern.
    jobs = []  # jobs[c][j] = (row, set_id, scale)
    for c in range(NCORES):
        rows = [ROWS_PER_CORE * c + i for i in range(ROWS_PER_CORE)]
        exp = {r: sorted(idx[r]) for r in rows}
        core_jobs = []
        for k in range(TOPK):
            for r in rows:
                e = int(exp[r][k])
                core_jobs.append((r, e, float(gated[r, e])))
        for r in rows:
            core_jobs.append((r, GEN, 1.0))
        jobs.append(core_jobs)

    # Core-uniform weight-load schedule: load before job j unless ALL cores
    # have set[j] == set[j-1].
    load_uniq, n_uniq = [], 0
    for j in range(JOBS_PER_CORE):
        dedupe = j > 0 and all(jobs[c][j][1] == jobs[c][j - 1][1]
                               for c in range(NCORES))
        if dedupe:
            load_uniq.append(None)
        else:
            load_uniq.append(n_uniq)
            n_uniq += 1

    nc = _build_nc(n_uniq, load_uniq)

    # Stage per-core inputs.
    in_maps = []
    for c in range(NCORES):
        core_jobs = jobs[c]
        w1_st = np.empty((n_uniq, D, DFF), np.float16)
        w2_st = np.empty((n_uniq, DFF, D), np.float16)
        for j, u in enumerate(load_uniq):
            if u is not None:
                s = core_jobs[j][1]
                w1_st[u] = w1s[s]
                w2_st[u] = w2s[s]
        xT_st = np.empty((ROWS_PER_CORE, D, L), np.float16)
        for i in range(ROWS_PER_CORE):
            xT_st[i] = x[ROWS_PER_CORE * c + i].T
        xr_st = np.empty((JOBS_PER_CORE, L, D), np.float16)
        b1_st = np.empty((128, JOBS_PER_CORE, MC1), np.float32)
        gb_st = np.empty((JOBS_PER_CORE, 2, D), np.float16)
        for j, (r, s, g) in enumerate(core_jobs):
            xr_st[j] = x[r] + b2s[s]
            b1_st[:, j, :] = b1s[s].reshape(MC1, 128).T
            gb_st[j, 0] = g * gms[s]
            gb_st[j, 1] = g * bts[s]
        in_maps.append({"w1": w1_st, "w2": w2_st, "xT": xT_st, "xr": xr_st,
                        "b1": b1_st, "gb": gb_st})

    res = bass_utils.run_bass_kernel_spmd(nc, in_maps, core_ids=list(range(NCORES)))
    global last_run
    last_run = res

    # Combine: out[r] = y_general + bf16(sum of gated expert outputs).
    import ml_dtypes
    out = np.empty((B, L, D), np.float32)
    for c in range(NCORES):
        y = res.results[c]["y"]
        for i in range(ROWS_PER_CORE):
            r = ROWS_PER_CORE * c + i
            comb = np.zeros((L, D), np.float32)
            gen = None
            for j, (jr, s, g) in enumerate(jobs[c]):
                if jr != r:
                    continue
                if s == GEN:
                    gen = y[j]
                else:
                    comb += y[j]
            out[r] = gen + comb.astype(ml_dtypes.bfloat16).astype(np.float32)
    return out



# revision 3
# speedup vs baseline: 1.3909x; 1.3909x over previous
"""Trainium2 Bass kernel for nn_IntraCycleMoELayer (MoE routing, 8 cores).

Strategy
--------
The reference computes all E=8 experts densely, but the top-2 gate zeroes all
but 2 experts per batch row.  Work that matters: per row, the top-1 routed
expert (gate >= 0.5 by softmax-renorm structure), the top-2 expert (often with
a near-zero gate), and the general expert, each an MLP block
  LN(gelu_tanh(x @ w1 + b1) @ w2 + b2 + x) * gamma + beta
over [L=512 tokens, D=768], DFF=3072.

Per-job precision schemes (picked for the 2e-2 rel-err budget; fp8e4 matmuls
use DoubleRow perf mode = 2x PE throughput, contracting K=256/instr):
  - general expert: all fp16 (accuracy anchor; weight-1 contribution).
  - top-1 expert ("M2"): mm1 fp16, gelu -> fp8 h, mm2 fp8 DoubleRow.  The
    w2 absmax scale is folded into the residual (LN is scale-invariant).
  - top-2 expert with gate >= 0.01 ("F8"): both matmuls fp8 DoubleRow;
    mm1 dequant is folded into the activation's scale operand.
  - top-2 expert with gate < 0.01: skipped (contributes < 1e-3 rel).

Layout: core c owns rows 2c, 2c+1: 2 general jobs + 2 M2 jobs (full 512
tokens) + the F8 jobs split into 256-token halves spread 1/core.  The gate
is folded into gamma/beta host-side; host sums per-row outputs (general +
bf16(sum of routed)) exactly as the reference does.
"""
import numpy as np
import ml_dtypes

import concourse.bass as bass
import concourse.mybir as mybir
import concourse.tile as tile
from concourse import bacc
from concourse.bass import ts
from concourse import bass_utils

B, L, D, DFF, DLLM, E, TOPK = 16, 512, 768, 3072, 4096, 8, 2
EPS_GATE = 1e-9
LN_EPS = 1e-5
NCORES = 8
RPC = 2                               # rows per core
KC1, MC1 = D // 128, DFF // 128       # 6, 24
KC2, TC = DFF // 128, L // 128        # 24, 4
HL = L // 2                           # 256-token half
F8NP = ml_dtypes.float8_e4m3          # TRN float8e4 (max 240)
F8CAP = 240.0
SKIP_G = 0.01
dt = mybir.dt

_cache = {}   # (nf8, m2_reload) -> finalized nc


def _router(cycle_numbers, DKP_embeddings, gate_We, gate_Wc, gate_b, gate_Wo,
            gate_bo):
    h = np.maximum(
        DKP_embeddings @ gate_We + cycle_numbers @ gate_Wc + gate_b, 0.0)
    logits = h @ gate_Wo + gate_bo                       # [B, E]
    idx = np.argsort(-logits, axis=1, kind="stable")[:, :TOPK]
    m = logits.max(axis=1, keepdims=True)
    p = np.exp(logits - m)
    p /= p.sum(axis=1, keepdims=True)
    mask = np.zeros_like(p)
    mask[np.arange(logits.shape[0])[:, None], idx] = 1.0
    gated = p * mask
    gated = gated / (gated.sum(axis=1, keepdims=True) + EPS_GATE)
    return idx, gated


def _build_nc(nf8, m2_reload):
    key = (nf8, m2_reload)
    if key in _cache:
        return _cache[key]

    n_w1a = 2 + (1 if m2_reload else 0)
    n_w2b = 1 + (1 if m2_reload else 0) + nf8
    n_w1b = max(nf8, 1)
    n_xtb = max(nf8, 1)
    NSLOT = 4 + nf8
    NYH = 8 + nf8

    nc = bacc.Bacc("TRN2", target_bir_lowering=False, debug=False)
    w1a_d = nc.dram_tensor("w1a", [n_w1a, D, DFF], dt.float16, kind="ExternalInput")
    w2a_d = nc.dram_tensor("w2a", [DFF, D], dt.float16, kind="ExternalInput")
    w1b_d = nc.dram_tensor("w1b", [n_w1b, D, DFF], dt.float8e4, kind="ExternalInput")
    w2b_d = nc.dram_tensor("w2b", [n_w2b, DFF, D], dt.float8e4, kind="ExternalInput")
    xTa_d = nc.dram_tensor("xTa", [RPC, D, L], dt.float16, kind="ExternalInput")
    xTb_d = nc.dram_tensor("xTb", [n_xtb, D, HL], dt.float8e4, kind="ExternalInput")
    xr_d = nc.dram_tensor("xr", [NYH, HL, D], dt.float16, kind="ExternalInput")
    b1_d = nc.dram_tensor("b1", [128, NSLOT, MC1], dt.float32, kind="ExternalInput")
    as_d = nc.dram_tensor("acts", [128, NSLOT], dt.float32, kind="ExternalInput")
    gb_d = nc.dram_tensor("gb", [NSLOT, 2, D], dt.float16, kind="ExternalInput")
    y_d = nc.dram_tensor("y", [NYH, HL, D], dt.float32, kind="ExternalOutput")

    gelu = mybir.ActivationFunctionType.Gelu_apprx_tanh
    DR = mybir.MatmulPerfMode.DoubleRow

    with tile.TileContext(nc) as tc, \
         tc.tile_pool(name="w1ap", bufs=1) as w1ap, \
         tc.tile_pool(name="w2ap", bufs=1) as w2ap, \
         tc.tile_pool(name="w1bp", bufs=1) as w1bp, \
         tc.tile_pool(name="w2bp", bufs=2) as w2bp, \
         tc.tile_pool(name="xtap", bufs=RPC) as xtap, \
         tc.tile_pool(name="xtbp", bufs=1) as xtbp, \
         tc.tile_pool(name="h16p", bufs=1) as h16p, \
         tc.tile_pool(name="h8p", bufs=1) as h8p, \
         tc.tile_pool(name="xrp", bufs=3) as xrp, \
         tc.tile_pool(name="gbp", bufs=2) as gbp, \
         tc.tile_pool(name="rp", bufs=3) as rp, \
         tc.tile_pool(name="sp", bufs=4) as sp, \
         tc.tile_pool(name="cp", bufs=1) as cp, \
         tc.tile_pool(name="php", bufs=4, space="PSUM") as php, \
         tc.tile_pool(name="pop", bufs=2, space="PSUM") as pop:

        from concourse.bass import _add_dep_helper

        eps_t = cp.tile([128, 1], dt.float32)
        nc.vector.memset(eps_t, LN_EPS)

        b1_all = cp.tile([128, NSLOT, MC1], dt.float32)
        nc.gpsimd.dma_start(b1_all, b1_d[:])
        as_all = cp.tile([128, NSLOT], dt.float32)
        nc.gpsimd.dma_start(as_all, as_d[:])

        # PE warmup on zeros: ramp the HAM clock gate while head DMAs fly.
        warm_z = cp.tile([128, 512], dt.float16)
        nc.vector.memset(warm_z, 0.0)
        for _ in range(32):
            wp_t = php.tile([128, 512], dt.float32, tag="ph")
            nc.tensor.matmul(wp_t, lhsT=warm_z[:, 0:128], rhs=warm_z,
                             start=True, stop=True)

        # xT row 0 per k-chunk on the critical path; row 1 deferred.
        xT_sb = []
        for _ in range(RPC):
            t = xtap.tile([128, KC1, L], dt.float16, tag="xT")
            xT_sb.append(t)
        xsrc0 = xTa_d[0].rearrange("(k p) l -> p k l", p=128)
        for k in range(KC1):
            nc.sync.dma_start(xT_sb[0][:, k, :], xsrc0[:, k, :])

        first_mm = None
        deferred = []

        w1a_sb = w2a_sb = w1b_sb = w2b_sb = xtb_sb = None
        n_w2b_used = 0
        yidx = 0
        for s in range(NSLOT):
            kind = "F16" if s < 2 else ("M2" if s < 4 else "F8")
            row = s % RPC if s < 4 else None

            # ---- weight loads ----
            if kind == "F16" and s == 0:
                w1a_sb = w1ap.tile([128, KC1, DFF], dt.float16, tag="w1a")
                src = w1a_d[0].rearrange("(k p) n -> p k n", p=128)
                H = DFF // 2
                for k in range(KC1):
                    nc.sync.dma_start(w1a_sb[:, k, 0:H], src[:, k, 0:H])
                for k in range(KC1):
                    nc.sync.dma_start(w1a_sb[:, k, H:DFF], src[:, k, H:DFF])
                w2a_sb = w2ap.tile([128, KC2, D], dt.float16, tag="w2a")
                w2src = w2a_d.rearrange("(k p) n -> p k n", p=128)
                for k in range(0, KC2, 6):
                    dma = nc.gpsimd.dma_start(w2a_sb[:, k:k + 6, :],
                                              w2src[:, k:k + 6, :])
                    deferred.append(dma)
            if kind == "M2" and (s == 2 or m2_reload):
                u = 1 if s == 2 else 2
                w1a_sb = w1ap.tile([128, KC1, DFF], dt.float16, tag="w1a")
                src = w1a_d[u].rearrange("(k p) n -> p k n", p=128)
                for k in range(KC1):
                    nc.sync.dma_start(w1a_sb[:, k, :], src[:, k, :])
                w2b_sb = w2bp.tile([128, KC2, D], dt.float8e4, tag="w2b")
                w2src = w2b_d[n_w2b_used].rearrange("(k p) n -> p k n", p=128)
                n_w2b_used += 1
                nc.gpsimd.dma_start(w2b_sb[:, 0:12, :], w2src[:, 0:12, :])
                nc.gpsimd.dma_start(w2b_sb[:, 12:KC2, :], w2src[:, 12:KC2, :])
            if kind == "F8":
                f = s - 4
                w1b_sb = w1bp.tile([128, KC1, DFF], dt.float8e4, tag="w1b")
                src = w1b_d[f].rearrange("(k p) n -> p k n", p=128)
                nc.gpsimd.dma_start(w1b_sb[:, 0:3, :], src[:, 0:3, :])
                nc.gpsimd.dma_start(w1b_sb[:, 3:KC1, :], src[:, 3:KC1, :])
                w2b_sb = w2bp.tile([128, KC2, D], dt.float8e4, tag="w2b")
                w2src = w2b_d[n_w2b_used].rearrange("(k p) n -> p k n", p=128)
                n_w2b_used += 1
                nc.gpsimd.dma_start(w2b_sb[:, 0:12, :], w2src[:, 0:12, :])
                nc.gpsimd.dma_start(w2b_sb[:, 12:KC2, :], w2src[:, 12:KC2, :])
                xtb_sb = xtbp.tile([128, KC1, HL], dt.float8e4, tag="xtb")
                xbsrc = xTb_d[f].rearrange("(k p) l -> p k l", p=128)
                nc.sync.dma_start(xtb_sb, xbsrc)

            # ---- per-slot small data ----
            gb_sb = gbp.tile([128, 2, D], dt.float16, tag="gb")
            gb_ap = gb_d[s]
            dma = nc.gpsimd.dma_start(gb_sb, bass.AP(tensor=gb_ap.tensor,
                                                     offset=gb_ap.offset,
                                                     ap=[[0, 128], *gb_ap.ap]))
            if s == 0:
                deferred.append(dma)
            nyh = 2 if kind != "F8" else 1
            xr_sb = []
            for hh in range(nyh):
                t_ = xrp.tile([128, 2, D], dt.float16, tag="xr")
                src = xr_d[yidx + hh].rearrange("(t p) d -> p t d", p=128)
                dma = nc.gpsimd.dma_start(t_, src)
                if s == 0:
                    deferred.append(dma)
                xr_sb.append(t_)
            if s == 0:
                src = xTa_d[1].rearrange("(k p) l -> p k l", p=128)
                for k in range(KC1):
                    deferred.append(
                        nc.sync.dma_start(xT_sb[1][:, k, :], src[:, k, :]))
            b1_sb = b1_all[:, s, :]
            as_sb = as_all[:, s:s + 1]

            ntc = TC if kind != "F8" else TC // 2

            # ---- mm1 + gelu ----
            if kind == "F8":
                h8 = h8p.tile([128, KC2, L], dt.float8e4, tag="h8")
                hdst, hw = h8, HL
                for m in range(MC1):
                    ph = php.tile([128, 512], dt.float32, tag="ph")
                    for k2 in range(KC1 // 2):
                        nc.tensor.matmul(
                            ph[:, 0:HL],
                            lhsT=w1b_sb[:, 2 * k2:2 * k2 + 2, ts(m, 128)],
                            rhs=xtb_sb[:, 2 * k2:2 * k2 + 2, :],
                            start=(k2 == 0), stop=(k2 == KC1 // 2 - 1),
                            perf_mode=DR)
                    nc.scalar.activation(out=h8[:, m, 0:HL], in_=ph[:, 0:HL],
                                         func=gelu, bias=b1_sb[:, m:m + 1],
                                         scale=as_sb)
            else:
                if kind == "F16":
                    h16 = h16p.tile([128, KC2, L], dt.float16, tag="h16")
                    hdst, hw = h16, L
                else:
                    h8 = h8p.tile([128, KC2, L], dt.float8e4, tag="h8")
                    hdst, hw = h8, L
                for m in range(MC1):
                    ph = php.tile([128, 512], dt.float32, tag="ph")
                    for k in range(KC1):
                        mm = nc.tensor.matmul(
                            ph, lhsT=w1a_sb[:, k, ts(m, 128)],
                            rhs=xT_sb[row][:, k, :],
                            start=(k == 0), stop=(k == KC1 - 1))
                        if first_mm is None and s == 0 and m == 12 and k == 0:
                            first_mm = mm
                            for dma in deferred:
                                _add_dep_helper(
                                    dma.ins, first_mm.ins, sync=True,
                                    reason="delay non-critical head DMA")
                    nc.scalar.activation(out=hdst[:, m, 0:hw], in_=ph[:, 0:hw],
                                         func=gelu, bias=b1_sb[:, m:m + 1],
                                         scale=as_sb)

            # ---- mm2 + residual + LN per 128-token chunk ----
            for t in range(ntc):
                po = pop.tile([128, D], dt.float32, tag="po")
                if kind == "F16":
                    for k in range(KC2):
                        nc.tensor.matmul(po[:, 0:512],
                                         lhsT=h16[:, k, ts(t, 128)],
                                         rhs=w2a_sb[:, k, 0:512],
                                         start=(k == 0), stop=(k == KC2 - 1))
                        nc.tensor.matmul(po[:, 512:D],
                                         lhsT=h16[:, k, ts(t, 128)],
                                         rhs=w2a_sb[:, k, 512:D],
                                         start=(k == 0), stop=(k == KC2 - 1))
                else:
                    for k2 in range(KC2 // 2):
                        nc.tensor.matmul(
                            po[:, 0:512],
                            lhsT=h8[:, 2 * k2:2 * k2 + 2, ts(t, 128)],
                            rhs=w2b_sb[:, 2 * k2:2 * k2 + 2, 0:512],
                            start=(k2 == 0), stop=(k2 == KC2 // 2 - 1),
                            perf_mode=DR)
                        nc.tensor.matmul(
                            po[:, 512:D],
                            lhsT=h8[:, 2 * k2:2 * k2 + 2, ts(t, 128)],
                            rhs=w2b_sb[:, 2 * k2:2 * k2 + 2, 512:D],
                            start=(k2 == 0), stop=(k2 == KC2 // 2 - 1),
                            perf_mode=DR)
                r_sb = rp.tile([128, D], dt.float32, tag="r")
                nc.vector.tensor_add(r_sb, po, xr_sb[t // 2][:, t % 2, :])
                stats = sp.tile([128, 3, 6], dt.float32, tag="st")
                for s3 in range(3):
                    nc.vector.bn_stats(stats[:, s3, :], r_sb[:, ts(s3, 256)])
                mv = sp.tile([128, 2], dt.float32, tag="mv")
                nc.vector.bn_aggr(mv, stats)
                rstd = sp.tile([128, 1], dt.float32, tag="rstd")
                nc.scalar.activation(out=rstd, in_=mv[:, 1:2],
                                     func=mybir.ActivationFunctionType.Sqrt,
                                     bias=eps_t, scale=1.0)
                nc.vector.reciprocal(rstd, rstd)
                nc.vector.tensor_scalar(out=r_sb, in0=r_sb, scalar1=mv[:, 0:1],
                                        scalar2=rstd,
                                        op0=mybir.AluOpType.subtract,
                                        op1=mybir.AluOpType.mult)
                nc.vector.tensor_mul(r_sb, r_sb, gb_sb[:, 0, :])
                nc.vector.tensor_add(r_sb, r_sb, gb_sb[:, 1, :])
                nc.sync.dma_start(
                    y_d[yidx + t // 2].rearrange("(t p) d -> p t d",
                                                 p=128)[:, t % 2, :], r_sb)
            yidx += nyh

    nc.finalize()
    _cache[key] = nc
    return nc


def kernel(cycle_curve_data, cycle_numbers, DKP_embeddings,
           gate_We, gate_Wc, gate_b, gate_Wo, gate_bo,
           e_w1, e_b1, e_w2, e_b2, e_gamma, e_beta,
           g_w1, g_b1, g_w2, g_b2, g_gamma, g_beta):
    x = np.asarray(cycle_curve_data, dtype=np.float32)
    idx, gated = _router(np.asarray(cycle_numbers, np.float32),
                         np.asarray(DKP_embeddings, np.float32),
                         np.asarray(gate_We, np.float32),
                         np.asarray(gate_Wc, np.float32),
                         np.asarray(gate_b, np.float32),
                         np.asarray(gate_Wo, np.float32),
                         np.asarray(gate_bo, np.float32))

    GEN = E
    w1s = {**{e: np.asarray(e_w1[e], np.float32) for e in range(E)},
           GEN: np.asarray(g_w1, np.float32)}
    w2s = {**{e: np.asarray(e_w2[e], np.float32) for e in range(E)},
           GEN: np.asarray(g_w2, np.float32)}
    b1s = {**{e: np.asarray(e_b1[e], np.float32) for e in range(E)},
           GEN: np.asarray(g_b1, np.float32)}
    b2s = {**{e: np.asarray(e_b2[e], np.float32) for e in range(E)},
           GEN: np.asarray(g_b2, np.float32)}
    gms = {**{e: np.asarray(e_gamma[e], np.float32) for e in range(E)},
           GEN: np.asarray(g_gamma, np.float32)}
    bts = {**{e: np.asarray(e_beta[e], np.float32) for e in range(E)},
           GEN: np.asarray(g_beta, np.float32)}

    # job classification
    m2_jobs = [(r, int(idx[r, 0]), float(gated[r, idx[r, 0]]))
               for r in range(B)]
    f8_jobs = [(r, int(idx[r, 1]), float(gated[r, idx[r, 1]]))
               for r in range(B) if gated[r, idx[r, 1]] >= SKIP_G]
    f8_halves = [(r, e, g, h) for (r, e, g) in f8_jobs for h in (0, 1)]
    nf8 = (len(f8_halves) + NCORES - 1) // NCORES
    m2_reload = any(m2_jobs[2 * c][1] != m2_jobs[2 * c + 1][1]
                    for c in range(NCORES))
    nc = _build_nc(nf8, m2_reload)

    n_w1a = 2 + (1 if m2_reload else 0)
    n_w2b = 1 + (1 if m2_reload else 0) + nf8
    n_w1b = max(nf8, 1)
    n_xtb = max(nf8, 1)
    NSLOT = 4 + nf8
    NYH = 8 + nf8

    # shared quantized weights (same content for every core that uses them)
    f16w = {}

    def w16(kind, s):
        if (kind, s) not in f16w:
            f16w[(kind, s)] = (w1s[s] if kind == 1 else w2s[s]).astype(np.float16)
        return f16w[(kind, s)]

    q8w = {}

    def w8(kind, s):
        if (kind, s) not in q8w:
            w = w1s[s] if kind == 1 else w2s[s]
            sc = F8CAP / max(float(np.abs(w).max()), 1e-30)
            q8w[(kind, s)] = ((w * sc).astype(F8NP), sc)
        return q8w[(kind, s)]

    # per-core F8 half assignment: core c takes halves [c::NCORES]
    f8_by_core = [[] for _ in range(NCORES)]
    for i, hf in enumerate(f8_halves):
        f8_by_core[i % NCORES].append(hf)

    in_maps = []
    slot_tables = []   # per core: list of (kind, row, expert, half, dummy)
    for c in range(NCORES):
        rows = [RPC * c + i for i in range(RPC)]
        w1a_st = np.empty((n_w1a, D, DFF), np.float16)
        w1a_st[0] = w16(1, GEN)
        w1a_st[1] = w16(1, m2_jobs[rows[0]][1])
        if m2_reload:
            w1a_st[2] = w16(1, m2_jobs[rows[1]][1])
        w2a_st = w16(2, GEN)
        w2b_st = np.zeros((n_w2b, DFF, D), F8NP)
        w1b_st = np.zeros((n_w1b, D, DFF), F8NP)
        xtb_st = np.zeros((n_xtb, D, HL), F8NP)
        xr_st = np.zeros((NYH, HL, D), np.float16)
        b1_st = np.zeros((128, NSLOT, MC1), np.float32)
        as_st = np.ones((128, NSLOT), np.float32)
        gb_st = np.zeros((NSLOT, 2, D), np.float16)
        xTa_st = np.empty((RPC, D, L), np.float16)
        for i, r in enumerate(rows):
            xTa_st[i] = x[r].T

        table = []
        n_w2b_used = 0
        yidx = 0
        for s in range(NSLOT):
            kind = "F16" if s < 2 else ("M2" if s < 4 else "F8")
            if kind == "F16":
                r = rows[s]
                b1_st[:, s, :] = b1s[GEN].reshape(MC1, 128).T
                gb_st[s, 0] = gms[GEN]
                gb_st[s, 1] = bts[GEN]
                xr = (x[r] + b2s[GEN]).astype(np.float16)
                xr_st[yidx] = xr[0:HL]
                xr_st[yidx + 1] = xr[HL:L]
                table.append((kind, r, GEN, None, False))
                yidx += 2
            elif kind == "M2":
                r, e, g = m2_jobs[rows[s - 2]]
                if s == 2 or m2_reload:
                    w2q, sw2 = w8(2, e)
                    w2b_st[n_w2b_used] = w2q
                    n_w2b_used += 1
                else:
                    _, sw2 = w8(2, e)
                b1_st[:, s, :] = b1s[e].reshape(MC1, 128).T
                gb_st[s, 0] = g * gms[e]
                gb_st[s, 1] = g * bts[e]
                xr = ((x[r] + b2s[e]) * sw2).astype(np.float16)
                xr_st[yidx] = xr[0:HL]
                xr_st[yidx + 1] = xr[HL:L]
                table.append((kind, r, e, None, False))
                yidx += 2
            else:
                f = s - 4
                if f < len(f8_by_core[c]):
                    r, e, g, h = f8_by_core[c][f]
                    w1q, sw1 = w8(1, e)
                    w2q, sw2 = w8(2, e)
                    w1b_st[f] = w1q
                    w2b_st[n_w2b_used] = w2q
                    xh = x[r, h * HL:(h + 1) * HL]
                    sx = F8CAP / max(float(np.abs(xh).max()), 1e-30)
                    xtb_st[f] = (xh.T * sx).astype(F8NP)
                    as_st[:, s] = 1.0 / (sx * sw1)
                    b1_st[:, s, :] = b1s[e].reshape(MC1, 128).T
                    gb_st[s, 0] = g * gms[e]
                    gb_st[s, 1] = g * bts[e]
                    xr_st[yidx] = ((xh + b2s[e]) * sw2).astype(np.float16)
                    table.append((kind, r, e, h, False))
                else:
                    table.append((kind, None, None, None, True))
                n_w2b_used += 1
                yidx += 1
        slot_tables.append(table)
        in_maps.append({"w1a": w1a_st, "w2a": w2a_st, "w1b": w1b_st,
                        "w2b": w2b_st, "xTa": xTa_st, "xTb": xtb_st,
                        "xr": xr_st, "b1": b1_st, "acts": as_st,
                        "gb": gb_st})

    res = bass_utils.run_bass_kernel_spmd(nc, in_maps,
                                          core_ids=list(range(NCORES)))
    global last_run
    last_run = res

    # Combine: out[r] = y_general + bf16(sum of gated expert outputs).
    gen = np.zeros((B, L, D), np.float32)
    comb = np.zeros((B, L, D), np.float32)
    for c in range(NCORES):
        y = res.results[c]["y"]
        yidx = 0
        for (kind, r, e, h, dummy) in slot_tables[c]:
            if kind in ("F16", "M2"):
                dst = gen if kind == "F16" else comb
                dst[r, 0:HL] += y[yidx]
                dst[r, HL:L] += y[yidx + 1]
                yidx += 2
            else:
                if not dummy:
                    comb[r, h * HL:(h + 1) * HL] += y[yidx]
                yidx += 1
    out = gen + comb.astype(ml_dtypes.bfloat16).astype(np.float32)
    return out


# revision 9
# speedup vs baseline: 1.6311x; 1.1727x over previous
"""Trainium2 Bass kernel for nn_IntraCycleMoELayer (MoE routing, 8 cores).

Strategy
--------
The reference computes all E=8 experts densely, but the top-2 gate zeroes all
but 2 experts per batch row.  Work that matters: per row, the top-1 routed
expert (gate >= 0.5 by softmax-renorm structure), the top-2 expert (often with
a near-zero gate), and the general expert, each an MLP block
  LN(gelu_tanh(x @ w1 + b1) @ w2 + b2 + x) * gamma + beta
over [L=512 tokens, D=768], DFF=3072.

Precision (for the 2e-2 rel-err budget; fp8e4 matmuls use DoubleRow perf
mode = 2x PE throughput, contracting K=256/instr):
  - general + top-1 expert ("M2"): mm1 fp16, gelu -> fp8 h, mm2 fp8
    DoubleRow.  The w2 absmax scale is folded into the residual (LN is
    scale-invariant up to eps, which is corrected via the Sqrt bias).
  - top-2 expert with gate >= 0.01 ("F8"): both matmuls fp8 DoubleRow; mm1
    dequant folds into the gelu activation's scale operand.
  - top-2 expert with gate < 0.01: skipped (contributes < 1e-3 rel).

When gamma is uniform and beta is zero (as in this model), the gate and
gamma fold into the LN rstd via the Sqrt activation's scale operand, so the
LN tail is a single tensor_scalar writing fp16 output.

Layout: core c owns rows 2c, 2c+1 -> 4 full M2 slots (general x2, top-1 x2);
the F8 jobs are split into 256-token halves spread 1 per core and run FIRST
(cheap fp8 weights shorten the critical head DMA); its mm2 is emitted after
the first M2 slot's mm1 so the gelu drain overlaps.  Host sums per-row
outputs (general + bf16(sum of routed)) exactly as the reference does.
"""
import numpy as np
import ml_dtypes

import concourse.bass as bass
import concourse.mybir as mybir
import concourse.tile as tile
from concourse import bacc
from concourse.bass import ts
from concourse import bass_utils

B, L, D, DFF, DLLM, E, TOPK = 16, 512, 768, 3072, 4096, 8, 2
EPS_GATE = 1e-9
LN_EPS = 1e-5
NCORES = 8
RPC = 2                               # rows per core
KC1, MC1 = D // 128, DFF // 128       # 6, 24
KC2, TC = DFF // 128, L // 128        # 24, 4
HL = L // 2                           # 256-token half
CB = DFF // 4                         # 768-column DMA piece
F8NP = ml_dtypes.float8_e4m3          # TRN float8e4 (max 240)
F8CAP = 240.0
SKIP_G = 0.01
dt = mybir.dt

_cache = {}   # (nf8, m2_reload, fold) -> finalized nc


def _router(cycle_numbers, DKP_embeddings, gate_We, gate_Wc, gate_b, gate_Wo,
            gate_bo):
    h = np.maximum(
        DKP_embeddings @ gate_We + cycle_numbers @ gate_Wc + gate_b, 0.0)
    logits = h @ gate_Wo + gate_bo                       # [B, E]
    idx = np.argsort(-logits, axis=1, kind="stable")[:, :TOPK]
    m = logits.max(axis=1, keepdims=True)
    p = np.exp(logits - m)
    p /= p.sum(axis=1, keepdims=True)
    mask = np.zeros_like(p)
    mask[np.arange(logits.shape[0])[:, None], idx] = 1.0
    gated = p * mask
    gated = gated / (gated.sum(axis=1, keepdims=True) + EPS_GATE)
    return idx, gated


def _slot_kinds(nf8, m2_reload):
    """Slot order shared by program build and host staging.

    Returns list of kinds: "F8" (half job, fp8 mm1+mm2) or "M2" (full job,
    fp16 mm1 + fp8 mm2).  w1a sets: [GEN, e6(row0), e6(row1) if reload].
    w2b sets are consumed in slot order by every slot's load.
    """
    kinds = []
    if nf8 >= 1:
        kinds.append("F8")
    kinds += ["M2", "M2", "M2", "M2"]
    kinds += ["F8"] * (nf8 - 1)
    return kinds


def _build_nc(nf8, m2_reload, fold):
    key = (nf8, m2_reload, fold)
    if key in _cache:
        return _cache[key]

    kinds = _slot_kinds(nf8, m2_reload)
    NSLOT = len(kinds)
    n_w1a = 2 + (1 if m2_reload else 0)
    # w2b sets in slot order: one per F8 slot, one for GEN (first M2),
    # one (or two with reload) for e6.
    n_w2b = nf8 + 2 + (1 if m2_reload else 0)
    n_w1b = max(nf8, 1)
    n_xtb = max(nf8, 1)
    NYH = 8 + nf8

    nc = bacc.Bacc("TRN2", target_bir_lowering=False, debug=False)
    w1a_d = nc.dram_tensor("w1a", [n_w1a, D, DFF], dt.float16, kind="ExternalInput")
    w1b_d = nc.dram_tensor("w1b", [n_w1b, D, DFF], dt.float8e4, kind="ExternalInput")
    w2b_d = nc.dram_tensor("w2b", [n_w2b, DFF, D], dt.float8e4, kind="ExternalInput")
    xTa_d = nc.dram_tensor("xTa", [RPC, D, L], dt.float16, kind="ExternalInput")
    xTb_d = nc.dram_tensor("xTb", [n_xtb, D, HL], dt.float8e4, kind="ExternalInput")
    xr_d = nc.dram_tensor("xr", [NYH, HL, D], dt.float16, kind="ExternalInput")
    b1_d = nc.dram_tensor("b1", [128, NSLOT, MC1], dt.float32, kind="ExternalInput")
    as_d = nc.dram_tensor("acts", [128, NSLOT, 3], dt.float32, kind="ExternalInput")
    gb_d = nc.dram_tensor("gb", [NSLOT, 2, D], dt.float16, kind="ExternalInput")
    y_d = nc.dram_tensor("y", [NYH, HL, D], dt.float16, kind="ExternalOutput")

    gelu = mybir.ActivationFunctionType.Gelu_apprx_tanh
    DR = mybir.MatmulPerfMode.DoubleRow

    with tile.TileContext(nc) as tc, \
         tc.tile_pool(name="w1ap", bufs=2) as w1ap, \
         tc.tile_pool(name="w1bp", bufs=1) as w1bp, \
         tc.tile_pool(name="w2bp", bufs=2) as w2bp, \
         tc.tile_pool(name="xtap", bufs=RPC) as xtap, \
         tc.tile_pool(name="xtbp", bufs=1) as xtbp, \
         tc.tile_pool(name="h8p", bufs=2) as h8p, \
         tc.tile_pool(name="xrp", bufs=3) as xrp, \
         tc.tile_pool(name="gbp", bufs=2) as gbp, \
         tc.tile_pool(name="rp", bufs=3) as rp, \
         tc.tile_pool(name="yp", bufs=3) as yp, \
         tc.tile_pool(name="sp", bufs=4) as sp, \
         tc.tile_pool(name="cp", bufs=1) as cp, \
         tc.tile_pool(name="php", bufs=4, space="PSUM") as php, \
         tc.tile_pool(name="pop", bufs=2, space="PSUM") as pop:

        # ---------- prologue: small loads + PE warmup ----------
        b1_all = cp.tile([128, NSLOT, MC1], dt.float32)
        nc.gpsimd.dma_start(b1_all, b1_d[:])
        as_all = cp.tile([128, NSLOT, 3], dt.float32)
        nc.gpsimd.dma_start(as_all, as_d[:])

        warm_z = cp.tile([128, 512], dt.float16)
        nc.vector.memset(warm_z, 0.0)
        for _ in range(32):
            wp_t = php.tile([128, 512], dt.float32, tag="ph")
            nc.tensor.matmul(wp_t, lhsT=warm_z[:, 0:128], rhs=warm_z,
                             start=True, stop=True)

        # ---------- prologue: critical-path weight streams ----------
        # sync queue: F8 weights first (small), then the second half of w1a
        # GEN + xT row1 + w1a e6 + w2b e6.
        # gpsimd queue: xT row0 + first half of w1a GEN + w2b e4/GEN.
        xtb_sb = None
        if nf8 >= 1:
            xtb_sb = xtbp.tile([128, KC1, HL], dt.float8e4, tag="xtb")
            nc.sync.dma_start(xtb_sb, xTb_d[0].rearrange("(k p) l -> p k l", p=128))
            w1b_sb = w1bp.tile([128, KC1, DFF], dt.float8e4, tag="w1b")
            w1bsrc = w1b_d[0].rearrange("(k p) n -> p k n", p=128)
            for cb in range(4):
                nc.sync.dma_start(w1b_sb[:, :, cb * CB:(cb + 1) * CB],
                                  w1bsrc[:, :, cb * CB:(cb + 1) * CB])

        xT_sb = []
        for _ in range(RPC):
            t_ = xtap.tile([128, KC1, L], dt.float16, tag="xT")
            xT_sb.append(t_)
        xa0 = xTa_d[0].rearrange("(k p) l -> p k l", p=128)
        nc.gpsimd.dma_start(xT_sb[0][:, 0:3, :], xa0[:, 0:3, :])
        nc.gpsimd.dma_start(xT_sb[0][:, 3:KC1, :], xa0[:, 3:KC1, :])

        w1a_gen = w1ap.tile([128, KC1, DFF], dt.float16, tag="w1a")
        w1asrc = w1a_d[0].rearrange("(k p) n -> p k n", p=128)
        for cb in range(2):
            nc.gpsimd.dma_start(w1a_gen[:, :, cb * CB:(cb + 1) * CB],
                                w1asrc[:, :, cb * CB:(cb + 1) * CB])
        for cb in range(2, 4):
            nc.sync.dma_start(w1a_gen[:, :, cb * CB:(cb + 1) * CB],
                              w1asrc[:, :, cb * CB:(cb + 1) * CB])
        xa1 = xTa_d[1].rearrange("(k p) l -> p k l", p=128)
        nc.sync.dma_start(xT_sb[1][:, 0:3, :], xa1[:, 0:3, :])
        nc.sync.dma_start(xT_sb[1][:, 3:KC1, :], xa1[:, 3:KC1, :])

        w1a_e6 = [None, None]
        w1a_e6[0] = w1ap.tile([128, KC1, DFF], dt.float16, tag="w1a", name="w1a_e6a")
        w1esrc = w1a_d[1].rearrange("(k p) n -> p k n", p=128)
        for cb in range(4):
            nc.sync.dma_start(w1a_e6[0][:, :, cb * CB:(cb + 1) * CB],
                              w1esrc[:, :, cb * CB:(cb + 1) * CB])
        if m2_reload:
            # Loaded lazily at its slot so the ring-WAR on the GEN buffer
            # doesn't block the sync queue head.
            w1a_e6[1] = None
        else:
            w1a_e6[1] = w1a_e6[0]

        # w2b sets, consumed in slot order.  e4 + GEN early on gpsimd; e6
        # late on sync (ring WAR on the e4 buffer resolves before needed).
        w2b_tiles = []
        n_w2b_emitted = 0

        def load_w2b(queue):
            nonlocal n_w2b_emitted
            t_ = w2bp.tile([128, KC2, D], dt.float8e4, tag="w2b")
            src = w2b_d[n_w2b_emitted].rearrange("(k p) n -> p k n", p=128)
            queue.dma_start(t_[:, 0:12, :], src[:, 0:12, :])
            queue.dma_start(t_[:, 12:KC2, :], src[:, 12:KC2, :])
            n_w2b_emitted += 1
            w2b_tiles.append(t_)
            return t_

        if nf8 >= 1:
            load_w2b(nc.gpsimd)            # e4 (slot 0)
        load_w2b(nc.gpsimd)                # GEN (M2 slots 0-1)
        load_w2b(nc.sync)                  # e6 (first M2e slot)
        if m2_reload:
            load_w2b(nc.sync)              # e6 second row

        kinds_ex = []   # (kind, slot_idx, yidx, row_or_f8idx, w1a_tile, w2b_tile)
        yidx = 0
        m2_seen = 0
        f8_seen = 0
        w2b_order = []
        if nf8 >= 1:
            w2b_order.append(0)
        w2b_order += [1 if nf8 >= 1 else 0] * 2
        base_e6 = (2 if nf8 >= 1 else 1)
        w2b_order += [base_e6, base_e6 + (1 if m2_reload else 0)]

        slot_descs = []
        for s, kind in enumerate(_slot_kinds(nf8, m2_reload)):
            if kind == "M2":
                row = m2_seen % RPC
                late = m2_seen == 3 and m2_reload
                w1a_t = w1a_gen if m2_seen < 2 else w1a_e6[row]
                w2b_t = w2b_tiles[w2b_order[s]]
                slot_descs.append(dict(kind=kind, s=s, yidx=yidx, row=row,
                                       w1a=w1a_t, w2b=w2b_t, late=late))
                m2_seen += 1
                yidx += 2
            else:
                slot_descs.append(dict(kind=kind, s=s, yidx=yidx, f8=f8_seen))
                f8_seen += 1
                yidx += 1

        # ---------- compute emission ----------
        def emit_mm1(sd):
            s = sd["s"]
            b1_sb = b1_all[:, s, :]
            gsc = as_all[:, s, 0:1]
            h8 = h8p.tile([128, KC2, L], dt.float8e4, tag="h8")
            sd["h8"] = h8
            if sd["kind"] == "F8":
                f = sd["f8"]
                if f == 0:
                    w1b_t, xtb_t = w1b_sb, xtb_sb
                else:
                    w1b_t = w1bp.tile([128, KC1, DFF], dt.float8e4, tag="w1b")
                    nc.sync.dma_start(
                        w1b_t, w1b_d[f].rearrange("(k p) n -> p k n", p=128))
                    xtb_t = xtbp.tile([128, KC1, HL], dt.float8e4, tag="xtb")
                    nc.sync.dma_start(
                        xtb_t, xTb_d[f].rearrange("(k p) l -> p k l", p=128))
                    sd["w2b"] = load_w2b(nc.gpsimd)
                for m in range(MC1):
                    ph = php.tile([128, 512], dt.float32, tag="ph")
                    for k2 in range(KC1 // 2):
                        nc.tensor.matmul(
                            ph[:, 0:HL],
                            lhsT=w1b_t[:, 2 * k2:2 * k2 + 2, ts(m, 128)],
                            rhs=xtb_t[:, 2 * k2:2 * k2 + 2, :],
                            start=(k2 == 0), stop=(k2 == KC1 // 2 - 1),
                            perf_mode=DR)
                    nc.scalar.activation(out=h8[:, m, 0:HL], in_=ph[:, 0:HL],
                                         func=gelu, bias=b1_sb[:, m:m + 1],
                                         scale=gsc)
            else:
                if sd.get("late") and sd["w1a"] is None:
                    w1a_t = w1ap.tile([128, KC1, DFF], dt.float16, tag="w1a")
                    src = w1a_d[2].rearrange("(k p) n -> p k n", p=128)
                    for cb in range(4):
                        nc.sync.dma_start(w1a_t[:, :, cb * CB:(cb + 1) * CB],
                                          src[:, :, cb * CB:(cb + 1) * CB])
                    sd["w1a"] = w1a_t
                w1a_t = sd["w1a"]
                xT_t = xT_sb[sd["row"]]
                for m in range(MC1):
                    ph = php.tile([128, 512], dt.float32, tag="ph")
                    for k in range(KC1):
                        nc.tensor.matmul(
                            ph, lhsT=w1a_t[:, k, ts(m, 128)],
                            rhs=xT_t[:, k, :],
                            start=(k == 0), stop=(k == KC1 - 1))
                    nc.scalar.activation(out=h8[:, m, :], in_=ph,
                                         func=gelu, bias=b1_sb[:, m:m + 1],
                                         scale=gsc)

        def emit_mm2_ln(sd):
            s = sd["s"]
            h8 = sd["h8"]
            if sd["kind"] == "F8" and sd["f8"] == 0:
                w2b_t = w2b_tiles[0]
            else:
                w2b_t = sd["w2b"]
            nyh = 2 if sd["kind"] == "M2" else 1
            xr_sb = []
            for hh in range(nyh):
                t_ = xrp.tile([128, 2, D], dt.float16, tag="xr")
                nc.gpsimd.dma_start(
                    t_, xr_d[sd["yidx"] + hh].rearrange("(t p) d -> p t d", p=128))
                xr_sb.append(t_)
            gb_sb = None
            if not fold:
                gb_sb = gbp.tile([128, 2, D], dt.float16, tag="gb")
                gb_ap = gb_d[s]
                nc.gpsimd.dma_start(gb_sb, bass.AP(tensor=gb_ap.tensor,
                                                   offset=gb_ap.offset,
                                                   ap=[[0, 128], *gb_ap.ap]))
            ntc = TC if sd["kind"] == "M2" else TC // 2
            for t in range(ntc):
                po = pop.tile([128, D], dt.float32, tag="po")
                for k2 in range(KC2 // 2):
                    nc.tensor.matmul(
                        po[:, 0:512],
                        lhsT=h8[:, 2 * k2:2 * k2 + 2, ts(t, 128)],
                        rhs=w2b_t[:, 2 * k2:2 * k2 + 2, 0:512],
                        start=(k2 == 0), stop=(k2 == KC2 // 2 - 1),
                        perf_mode=DR)
                    nc.tensor.matmul(
                        po[:, 512:D],
                        lhsT=h8[:, 2 * k2:2 * k2 + 2, ts(t, 128)],
                        rhs=w2b_t[:, 2 * k2:2 * k2 + 2, 512:D],
                        start=(k2 == 0), stop=(k2 == KC2 // 2 - 1),
                        perf_mode=DR)
                r_sb = rp.tile([128, D], dt.float32, tag="r")
                nc.vector.tensor_add(r_sb, po, xr_sb[t // 2][:, t % 2, :])
                stats = sp.tile([128, 3, 6], dt.float32, tag="st")
                for s3 in range(3):
                    nc.vector.bn_stats(stats[:, s3, :], r_sb[:, ts(s3, 256)])
                mv = sp.tile([128, 2], dt.float32, tag="mv")
                nc.vector.bn_aggr(mv, stats)
                rstd = sp.tile([128, 1], dt.float32, tag="rstd")
                nc.scalar.activation(out=rstd, in_=mv[:, 1:2],
                                     func=mybir.ActivationFunctionType.Sqrt,
                                     bias=as_all[:, s, 2:3],
                                     scale=as_all[:, s, 1:2])
                nc.vector.reciprocal(rstd, rstd)
                y16 = yp.tile([128, D], dt.float16, tag="y16")
                if fold:
                    nc.vector.tensor_scalar(out=y16, in0=r_sb,
                                            scalar1=mv[:, 0:1], scalar2=rstd,
                                            op0=mybir.AluOpType.subtract,
                                            op1=mybir.AluOpType.mult)
                else:
                    nc.vector.tensor_scalar(out=r_sb, in0=r_sb,
                                            scalar1=mv[:, 0:1], scalar2=rstd,
                                            op0=mybir.AluOpType.subtract,
                                            op1=mybir.AluOpType.mult)
                    nc.vector.tensor_mul(r_sb, r_sb, gb_sb[:, 0, :])
                    nc.vector.tensor_add(y16, r_sb, gb_sb[:, 1, :])
                nc.sync.dma_start(
                    y_d[sd["yidx"] + t // 2].rearrange(
                        "(t p) d -> p t d", p=128)[:, t % 2, :], y16)

        if nf8 >= 1:
            # F8 mm1 first (cheap weights), then first M2 mm1 so the PE is
            # fed while the F8 gelu drains; F8 mm2 slots in after.
            emit_mm1(slot_descs[0])
            emit_mm1(slot_descs[1])
            emit_mm2_ln(slot_descs[0])
            emit_mm2_ln(slot_descs[1])
            rest = slot_descs[2:]
        else:
            rest = slot_descs
        for sd in rest:
            emit_mm1(sd)
            emit_mm2_ln(sd)

    nc.finalize()
    _cache[key] = nc
    return nc


def kernel(cycle_curve_data, cycle_numbers, DKP_embeddings,
           gate_We, gate_Wc, gate_b, gate_Wo, gate_bo,
           e_w1, e_b1, e_w2, e_b2, e_gamma, e_beta,
           g_w1, g_b1, g_w2, g_b2, g_gamma, g_beta):
    x = np.asarray(cycle_curve_data, dtype=np.float32)
    idx, gated = _router(np.asarray(cycle_numbers, np.float32),
                         np.asarray(DKP_embeddings, np.float32),
                         np.asarray(gate_We, np.float32),
                         np.asarray(gate_Wc, np.float32),
                         np.asarray(gate_b, np.float32),
                         np.asarray(gate_Wo, np.float32),
                         np.asarray(gate_bo, np.float32))

    GEN = E
    w1s = {**{e: np.asarray(e_w1[e], np.float32) for e in range(E)},
           GEN: np.asarray(g_w1, np.float32)}
    w2s = {**{e: np.asarray(e_w2[e], np.float32) for e in range(E)},
           GEN: np.asarray(g_w2, np.float32)}
    b1s = {**{e: np.asarray(e_b1[e], np.float32) for e in range(E)},
           GEN: np.asarray(g_b1, np.float32)}
    b2s = {**{e: np.asarray(e_b2[e], np.float32) for e in range(E)},
           GEN: np.asarray(g_b2, np.float32)}
    gms = {**{e: np.asarray(e_gamma[e], np.float32) for e in range(E)},
           GEN: np.asarray(g_gamma, np.float32)}
    bts = {**{e: np.asarray(e_beta[e], np.float32) for e in range(E)},
           GEN: np.asarray(g_beta, np.float32)}

    # job classification (top-1 always has gate >= 0.5 -> M2; top-2 skipped
    # below SKIP_G, else F8 halves)
    m2_jobs = [(r, int(idx[r, 0]), float(gated[r, idx[r, 0]]))
               for r in range(B)]
    f8_jobs = [(r, int(idx[r, 1]), float(gated[r, idx[r, 1]]))
               for r in range(B) if gated[r, idx[r, 1]] >= SKIP_G]
    f8_halves = [(r, e, g, h) for (r, e, g) in f8_jobs for h in (0, 1)]
    nf8 = (len(f8_halves) + NCORES - 1) // NCORES
    m2_reload = any(m2_jobs[2 * c][1] != m2_jobs[2 * c + 1][1]
                    for c in range(NCORES))

    used_sets = {GEN} | {e for _, e, _ in m2_jobs} | {e for _, e, _ in f8_jobs}
    fold = all(
        np.all(gms[s] == gms[s].flat[0]) and gms[s].flat[0] > 0
        and np.all(bts[s] == 0.0) for s in used_sets)

    nc = _build_nc(nf8, m2_reload, fold)

    kinds = _slot_kinds(nf8, m2_reload)
    NSLOT = len(kinds)
    n_w1a = 2 + (1 if m2_reload else 0)
    n_w2b = nf8 + 2 + (1 if m2_reload else 0)
    n_w1b = max(nf8, 1)
    n_xtb = max(nf8, 1)
    NYH = 8 + nf8

    f16w, q8w = {}, {}

    def w16(s):
        if s not in f16w:
            f16w[s] = w1s[s].astype(np.float16)
        return f16w[s]

    def w8(kind, s):
        if (kind, s) not in q8w:
            w = w1s[s] if kind == 1 else w2s[s]
            sc = F8CAP / max(float(np.abs(w).max()), 1e-30)
            q8w[(kind, s)] = ((w * sc).astype(F8NP), sc)
        return q8w[(kind, s)]

    f8_by_core = [[] for _ in range(NCORES)]
    for i, hf in enumerate(f8_halves):
        f8_by_core[i % NCORES].append(hf)

    in_maps = []
    slot_tables = []   # per core: list of (kind, row, expert, half, dummy)
    for c in range(NCORES):
        rows = [RPC * c + i for i in range(RPC)]
        w1a_st = np.empty((n_w1a, D, DFF), np.float16)
        w1a_st[0] = w16(GEN)
        w1a_st[1] = w16(m2_jobs[rows[0]][1])
        if m2_reload:
            w1a_st[2] = w16(m2_jobs[rows[1]][1])
        w2b_st = np.zeros((n_w2b, DFF, D), F8NP)
        w1b_st = np.zeros((n_w1b, D, DFF), F8NP)
        xtb_st = np.zeros((n_xtb, D, HL), F8NP)
        xr_st = np.zeros((NYH, HL, D), np.float16)
        b1_st = np.zeros((128, NSLOT, MC1), np.float32)
        as_st = np.ones((128, NSLOT, 3), np.float32)
        as_st[:, :, 2] = LN_EPS
        gb_st = np.zeros((NSLOT, 2, D), np.float16)
        xTa_st = np.empty((RPC, D, L), np.float16)
        for i, r in enumerate(rows):
            xTa_st[i] = x[r].T

        table = []
        n_w2b_used = 0
        m2_seen = 0
        f8_seen = 0
        yidx = 0
        for s, kind in enumerate(kinds):
            if kind == "M2":
                row = rows[m2_seen % RPC]
                e = GEN if m2_seen < 2 else m2_jobs[row][1]
                g = 1.0 if m2_seen < 2 else m2_jobs[row][2]
                load = (m2_seen == 0) or (m2_seen == 2) or \
                    (m2_seen == 3 and m2_reload)
                if load:
                    w2q, sw2 = w8(2, e)
                    w2b_st[n_w2b_used] = w2q
                    n_w2b_used += 1
                else:
                    _, sw2 = w8(2, e)
                b1_st[:, s, :] = b1s[e].reshape(MC1, 128).T
                gam = float(gms[e].flat[0]) if fold else 1.0
                as_st[:, s, 1] = 1.0 / (g * gam) ** 2 if fold else 1.0
                as_st[:, s, 2] = LN_EPS * sw2 ** 2 / ((g * gam) ** 2 if fold else 1.0)
                gb_st[s, 0] = g * gms[e]
                gb_st[s, 1] = g * bts[e]
                xr = ((x[row] + b2s[e]) * sw2).astype(np.float16)
                xr_st[yidx] = xr[0:HL]
                xr_st[yidx + 1] = xr[HL:L]
                table.append((kind, row, e, None, False))
                m2_seen += 1
                yidx += 2
            else:
                f = f8_seen
                f8_seen += 1
                if f < len(f8_by_core[c]):
                    r, e, g, h = f8_by_core[c][f]
                    w1q, sw1 = w8(1, e)
                    w2q, sw2 = w8(2, e)
                    w1b_st[f] = w1q
                    w2b_st[n_w2b_used] = w2q
                    xh = x[r, h * HL:(h + 1) * HL]
                    sx = F8CAP / max(float(np.abs(xh).max()), 1e-30)
                    xtb_st[f] = (xh.T * sx).astype(F8NP)
                    as_st[:, s, 0] = 1.0 / (sx * sw1)
                    gam = float(gms[e].flat[0]) if fold else 1.0
                    as_st[:, s, 1] = 1.0 / (g * gam) ** 2 if fold else 1.0
                    as_st[:, s, 2] = LN_EPS * sw2 ** 2 / ((g * gam) ** 2 if fold else 1.0)
                    b1_st[:, s, :] = b1s[e].reshape(MC1, 128).T
                    gb_st[s, 0] = g * gms[e]
                    gb_st[s, 1] = g * bts[e]
                    xr_st[yidx] = ((xh + b2s[e]) * sw2).astype(np.float16)
                    table.append((kind, r, e, h, False))
                else:
                    table.append((kind, None, None, None, True))
                n_w2b_used += 1
                yidx += 1
        slot_tables.append(table)
        in_maps.append({"w1a": w1a_st, "w1b": w1b_st, "w2b": w2b_st,
                        "xTa": xTa_st, "xTb": xtb_st, "xr": xr_st,
                        "b1": b1_st, "acts": as_st, "gb": gb_st})

    res = bass_utils.run_bass_kernel_spmd(nc, in_maps,
                                          core_ids=list(range(NCORES)))
    global last_run
    last_run = res

    # Combine: out[r] = y_general + bf16(sum of gated expert outputs).
    gen = np.zeros((B, L, D), np.float32)
    comb = np.zeros((B, L, D), np.float32)
    for c in range(NCORES):
        y = res.results[c]["y"].astype(np.float32)
        yidx = 0
        m2_seen = 0
        for (kind, r, e, h, dummy) in slot_tables[c]:
            if kind == "M2":
                dst = gen if m2_seen < 2 else comb
                dst[r, 0:HL] += y[yidx]
                dst[r, HL:L] += y[yidx + 1]
                m2_seen += 1
                yidx += 2
            else:
                if not dummy:
                    comb[r, h * HL:(h + 1) * HL] += y[yidx]
                yidx += 1
    out = gen + comb.astype(ml_dtypes.bfloat16).astype(np.float32)
    return out
